# revision 17
# baseline (speedup 1.0000x reference)
"""ClassAttention (decode-style single-query attention) on 8 TRN2 NeuronCores.

Math (per batch b):
    kv = x @ Wkv              # [N, 2*H*D], k half cols 0:1024, v half 1024:2048
    q  = x[0] @ Wq            # [H*D]  (CLS token only)
    logits[t, h] = scale * sum_d q[h,d] * k[t, h*64+d]
    attn = softmax_t(logits)
    cls[h,d] = sum_t attn[t,h] * v[t, h*64+d]
    out = cls @ Wproj + bproj

v3 restructuring (v2 was PE-issue-bound: 3300 tiny matmuls):
  - All sweep matmuls are N=512 with TINY stationaries:
      logits^T[h, t] : lhsT = wkf block [128, 16], moving = x^T [128, 512]
      r[h, c]        : lhsT = e block   [128, 16], moving = x   [128, 512]
  - x^T comes from the DMA XBAR (SBUF->SBUF transposing DMA, one call per
    1024-row chunk), alternating between the sync and scalar HWDGE queues
    so the transposes of consecutive chunks overlap.
  - Attention accumulation lives in PSUM chains (64 matmuls/batch).
  - sum_t exp is deferred to batch end (DVE reduces over the kept e tiles)
    so no serial DVE chain sits in the sweep's critical path.
  - Weights: Wq/Wk f32 on the scalar queue (consumed by the q-fold, freed
    after); Wv/Wproj stream f32 through a small staging pool mid-kernel
    (split across both HWDGE queues) and are cast to bf16 on the ACT
    engine while the sweeps run.
  - Finalize is split: the weight-independent part (r scale, sums) runs at
    sweep end; the Wv/Wproj part is emitted after the next batch's second
    chunk so it overlaps that batch's sweep.
  - Softmax still runs without max-subtraction (logits are O(1)); the
    1/sum(exp) normalization is applied to the tiny r[h, c] tensor.

Sharding: pure data-parallel over B: 16 batches / 8 cores = 2 per core.
Weights are replicated; each core returns its [2, 1024] output shard.
"""

import numpy as np

import concourse.bass as bass
import concourse.mybir as mybir
import concourse.tile as tile
from concourse import bacc
from concourse.bass_utils import run_bass_kernel_spmd
from concourse.masks import make_identity

F32 = mybir.dt.float32
BF16 = mybir.dt.bfloat16

B, SEQ, C = 16, 4096, 1024
H, D = 16, 64
SCALE = D ** -0.5  # 0.125
N_CORES = 8
BPC = B // N_CORES          # batches per core
CB = C // 128               # 8 contraction blocks
RPC = 1024                  # seq rows per chunk
NCH = SEQ // RPC            # 4 chunks per batch
SUB = RPC // 128            # 8 sub-tiles (of 128 rows) per chunk


def _build():
    nc = bacc.Bacc(
        "TRN2", target_bir_lowering=False, debug=False, num_devices=N_CORES
    )
    x_ap = nc.dram_tensor("x", [BPC, SEQ, C], F32, kind="ExternalInput").ap()
    wq_ap = nc.dram_tensor("Wq", [C, H * D], F32, kind="ExternalInput").ap()
    wkv_ap = nc.dram_tensor("Wkv", [C, 2 * H * D], F32, kind="ExternalInput").ap()
    wp_ap = nc.dram_tensor("Wproj", [H * D, C], F32, kind="ExternalInput").ap()
    bp_ap = nc.dram_tensor("bproj", [C], F32, kind="ExternalInput").ap()
    out_ap = nc.dram_tensor("out", [BPC, C], F32, kind="ExternalOutput").ap()

    with tile.TileContext(nc) as tc:
        _emit(nc, tc, x_ap, wq_ap, wkv_ap, wp_ap, bp_ap, out_ap)
    nc.compile()
    return nc


def _emit_qfold(nc, tc, consts, wstage, wq_ap, wkv_ap, xcls_f, sc_row,
                wkf_bf, qb_sb):
    """q = x_cls @ Wq; wkf[b][c, g, h] = scale * sum_d q[b,(h,d)] * Wk[c,(h,d)].

    Runs inside the DMA-bound prefix (PE and DVE are otherwise idle there).
    Wq/Wk stream block-by-block through the shared staging pool on the
    scalar HWDGE queue — no 8 MB resident copy."""
    with (
        tc.tile_pool(name="fold", bufs=2) as fold_pool,
        tc.tile_pool(name="qps", bufs=4, space="PSUM") as qps,
        tc.tile_pool(name="qbps", bufs=1, space="PSUM") as qbps,
    ):
        q_sb = [
            consts.tile([1, H * D], F32, tag=f"q{b}", name=f"q{b}")
            for b in range(BPC)
        ]
        q_ps = {}
        for b in range(BPC):
            for ch in range(2):
                qp = qps.tile([1, 512], F32, tag="q", name=f"qps{b}{ch}")
                q_ps[(b, ch)] = qp
        for g in range(CB):
            wst = wstage.tile([128, 1024], F32, tag="wst")
            nc.scalar.dma_start(wst[:], wq_ap[g * 128 : (g + 1) * 128, :])
            for b in range(BPC):
                for ch in range(2):
                    nc.tensor.matmul(
                        q_ps[(b, ch)][:],
                        xcls_f[:, g, b : b + 1],
                        wst[:, ch * 512 : (ch + 1) * 512],
                        start=(g == 0),
                        stop=(g == CB - 1),
                    )
        for b in range(BPC):
            for ch in range(2):
                nc.vector.tensor_copy(
                    q_sb[b][:, ch * 512 : (ch + 1) * 512], q_ps[(b, ch)][:]
                )

        # qb[b][c_p, hd] = scale * q[b, hd] broadcast down partitions
        for b in range(BPC):
            for ch in range(2):
                qb_ps = qbps.tile([128, 512], F32, tag="qb")
                nc.tensor.matmul(
                    qb_ps[:],
                    sc_row[0:1, :],
                    q_sb[b][0:1, ch * 512 : (ch + 1) * 512],
                    start=True,
                    stop=True,
                )
                nc.vector.tensor_copy(
                    qb_sb[b][:, ch * 512 : (ch + 1) * 512], qb_ps[:]
                )

        for g in range(CB):
            wst = wstage.tile([128, 1024], F32, tag="wst")
            nc.scalar.dma_start(
                wst[:], wkv_ap[g * 128 : (g + 1) * 128, 0:1024]
            )
            for b in range(BPC):
                prod = fold_pool.tile([128, H * D], F32, tag="prod")
                nc.vector.tensor_mul(prod[:], wst[:], qb_sb[b][:])
                wkf_g = fold_pool.tile([128, H], F32, tag="wkfg")
                nc.vector.tensor_reduce(
                    wkf_g[:].unsqueeze(2),
                    prod[:].rearrange("p (h d) -> p h d", d=D),
                    axis=mybir.AxisListType.X,
                    op=mybir.AluOpType.add,
                )
                nc.vector.tensor_copy(wkf_bf[b][:, g, :], wkf_g[:])


class _BatchState:
    def __init__(self):
        self.r_psA = None
        self.r_psB = None
        self.es = []        # 8 e_sb tiles per batch, kept for end-sums
        self.xts = {}
        self.pend = None
        self.r_sb = None
        self.rec = None


def _emit(nc, tc, x_ap, wq_ap, wkv_ap, wp_ap, bp_ap, out_ap):
    with tc.tile_pool(name="consts", bufs=1) as consts:
        wv_bf = consts.tile([128, CB, 1024], BF16)
        wp_bf = consts.tile([128, CB, 1024], BF16)

        bproj_sb = consts.tile([1, C], F32)
        nc.scalar.dma_start(bproj_sb[:], bp_ap[:].unsqueeze(0))

        xcls_f = consts.tile([128, CB, BPC], F32)
        for b in range(BPC):
            nc.scalar.dma_start(
                xcls_f[:, :, b : b + 1],
                x_ap[b, 0:1, :].rearrange("o (g p) -> p g o", p=128),
            )

        sc_row = consts.tile([1, 128], F32)
        nc.vector.memset(sc_row[:], SCALE)
        id16_bf = consts.tile([16, 16], BF16)
        make_identity(nc, id16_bf[:])

        wkf_bf = [
            consts.tile([128, CB, H], BF16, tag=f"wkf{b}", name=f"wkf{b}")
            for b in range(BPC)
        ]
        qb_sb = [
            consts.tile([128, C], F32, tag=f"qb{b}", name=f"qb{b}")
            for b in range(BPC)
        ]

        with (
            tc.tile_pool(name="xbf", bufs=2) as xbf_pool,
            tc.tile_pool(name="xt", bufs=2) as xt_pool,
            tc.tile_pool(name="wstage", bufs=2) as wstage,
            tc.tile_pool(name="esb", bufs=18) as esb_pool,
            tc.tile_pool(name="ebf", bufs=4) as ebf_pool,
            tc.tile_pool(name="small", bufs=2) as small,
        ):
            # ---- SWDGE queue: x chunk cast-loads only ----
            x_tiles = {}
            for b in range(BPC):
                for k in range(NCH):
                    x_bf = xbf_pool.tile([128, SUB, 1024], BF16, tag="x")
                    nc.gpsimd.dma_start(
                        x_bf[:],
                        x_ap[b, k * RPC : (k + 1) * RPC, :].rearrange(
                            "(p i) c -> p i c", p=128
                        ),
                    )
                    x_tiles[(b, k)] = x_bf

            # qfold's PSUM pools close before the sweep's PSUM pools open
            _emit_qfold(nc, tc, consts, wstage, wq_ap, wkv_ap, xcls_f, sc_row,
                        wkf_bf, qb_sb)

            sts = [_BatchState() for _ in range(BPC)]

            sweep_psum = tc.tile_pool(name="lgps", bufs=3, space="PSUM")
            lgps = sweep_psum.__enter__()
            xat_psum = tc.tile_pool(name="xatps", bufs=2, space="PSUM")
            xatps = xat_psum.__enter__()
            t16_psum = tc.tile_pool(name="t16ps", bufs=2, space="PSUM")
            t16ps = t16_psum.__enter__()

            def emit_xt(b, k):
                xt = xt_pool.tile([128, SUB * CB, 128], BF16, tag="xt")
                eng = nc.sync if k % 2 == 0 else nc.scalar
                eng.dma_start(xt[:], x_tiles[(b, k)][:], transpose=True)
                sts[b].xts[k] = xt

            def emit_logits(b, k):
                st = sts[b]
                xt = st.xts[k]
                es = []
                for grp in range(2):
                    lg = lgps.tile([16, 512], F32, tag="lg")
                    m0 = grp * 4 * CB
                    for g in range(CB):
                        nc.tensor.matmul(
                            lg[:],
                            wkf_bf[b][:, g, :],
                            xt[:, m0 + g : m0 + g + 3 * CB + 1 : CB, :],
                            start=(g == 0),
                            stop=(g == CB - 1),
                        )
                    e_sb = esb_pool.tile([16, 512], BF16, tag="e")
                    nc.scalar.activation(
                        e_sb[:], lg[:], mybir.ActivationFunctionType.Exp
                    )
                    es.append(e_sb)
                st.es.extend(es)
                st.pend = (k, es)

            def emit_racc(b):
                st = sts[b]
                if st.pend is None:
                    return
                k, es = st.pend
                st.pend = None
                x_bf = x_tiles[(b, k)]
                if st.r_psA is None:
                    st.r_psA = xatps.tile([16, 512], F32, tag="xat", name="rA")
                    st.r_psB = xatps.tile([16, 512], F32, tag="xat", name="rB")
                # all 8 eT transposes into one PSUM tile, one DVE copy out
                eT = t16ps.tile([128, SUB * H], BF16, tag="e16")
                for grp in range(2):
                    for j in range(4):
                        i = grp * 4 + j
                        nc.tensor.transpose(
                            eT[:, i * H : (i + 1) * H],
                            es[grp][:, j * 128 : (j + 1) * 128],
                            id16_bf[:],
                        )
                e_bf = ebf_pool.tile([128, SUB * H], BF16, tag="ebf")
                nc.vector.tensor_copy(e_bf[:], eT[:])
                for i in range(SUB):
                    first = k == 0 and i == 0
                    last = k == NCH - 1 and i == SUB - 1
                    nc.tensor.matmul(
                        st.r_psA[:], e_bf[:, i * H : (i + 1) * H],
                        x_bf[:, i, 0:512], start=first, stop=last,
                    )
                    nc.tensor.matmul(
                        st.r_psB[:], e_bf[:, i * H : (i + 1) * H],
                        x_bf[:, i, 512:1024], start=first, stop=last,
                    )

            def sweep_chunks(b, ks):
                """Depth-1 software pipeline: racc(k-1) is emitted after
                logits(k), so the PE never waits on the ACT exp."""
                st = sts[b]
                for k in ks:
                    if k == 0:
                        emit_xt(b, 0)
                    prev = st.pend
                    emit_logits(b, k)   # sets st.pend = (k, es)
                    cur = st.pend
                    st.pend = prev
                    if k + 1 < NCH:
                        emit_xt(b, k + 1)
                    emit_racc(b)        # racc for chunk k-1 (if any)
                    st.pend = cur

            def fin_part1(b):
                """Weight-independent: scale r by 1/sum(exp); frees PSUM."""
                st = sts[b]
                sums_all = small.tile([16, CB], F32, tag=f"sall{b}", name=f"sall{b}")
                for gi, e_sb in enumerate(st.es):
                    nc.vector.tensor_reduce(
                        sums_all[:, gi : gi + 1], e_sb[:],
                        axis=mybir.AxisListType.X, op=mybir.AluOpType.add,
                    )
                sums = small.tile([16, 1], F32, tag=f"sums{b}", name=f"sums{b}")
                nc.vector.tensor_reduce(
                    sums[:], sums_all[:],
                    axis=mybir.AxisListType.X, op=mybir.AluOpType.add,
                )
                rec = small.tile([16, 1], F32, tag=f"rec{b}", name=f"rec{b}")
                nc.vector.reciprocal(rec[:], sums[:])
                r_sb = small.tile([16, C], F32, tag=f"rsb{b}", name=f"rsb{b}")
                nc.vector.tensor_scalar_mul(r_sb[:, 0:512], st.r_psA[:], rec[:])
                nc.vector.tensor_scalar_mul(r_sb[:, 512:1024], st.r_psB[:], rec[:])
                st.r_sb = r_sb

            def fin_part2(b):
                """Needs wv_bf / wp_bf."""
                st = sts[b]
                r_bf = small.tile([16, C], BF16, tag=f"rbf{b}", name=f"rbf{b}")
                nc.vector.tensor_copy(r_bf[:], st.r_sb[:])
                rT_ps = t16ps.tile([128, CB * H], BF16, tag="e16")
                for g in range(CB):
                    nc.tensor.transpose(
                        rT_ps[:, g * H : (g + 1) * H],
                        r_bf[:, g * 128 : (g + 1) * 128],
                        id16_bf[:],
                    )
                rT_bf = small.tile([128, CB, H], BF16, tag=f"rT{b}", name=f"rT{b}")
                nc.vector.tensor_copy(
                    rT_bf[:].rearrange("p g h -> p (g h)"), rT_ps[:]
                )

                cls_bf = small.tile([16, C], BF16, tag=f"cls{b}", name=f"cls{b}")
                for ch in range(2):
                    cls_ps = lgps.tile([16, 512], F32, tag="lg")
                    for g in range(CB):
                        nc.tensor.matmul(
                            cls_ps[:],
                            rT_bf[:, g, :],
                            wv_bf[:, g, ch * 512 : (ch + 1) * 512],
                            start=(g == 0),
                            stop=(g == CB - 1),
                        )
                    nc.vector.tensor_copy(
                        cls_bf[:, ch * 512 : (ch + 1) * 512], cls_ps[:]
                    )

                # diagonal pick: clsv[hd] = cls_bf[hd//64, hd]
                aT = t16ps.tile([128, CB * H], BF16, tag="e16")
                for g in range(CB):
                    nc.tensor.transpose(
                        aT[:, g * H : (g + 1) * H],
                        cls_bf[:, g * 128 : (g + 1) * 128],
                        id16_bf[:],
                    )
                clsv_bf = small.tile([128, CB], BF16, tag=f"cv{b}", name=f"cv{b}")
                for g in range(CB):
                    for half in range(2):
                        rows = slice(64 * half, 64 * half + 64)
                        nc.vector.tensor_copy(
                            clsv_bf[rows, g : g + 1],
                            aT[rows, g * H + 2 * g + half : g * H + 2 * g + half + 1],
                        )

                o_sb = small.tile([1, C], F32, tag=f"osb{b}", name=f"osb{b}")
                for ch in range(2):
                    o_ps = lgps.tile([16, 512], F32, tag="lg")
                    for g in range(CB):
                        nc.tensor.matmul(
                            o_ps[0:1, :],
                            clsv_bf[:, g : g + 1],
                            wp_bf[:, g, ch * 512 : (ch + 1) * 512],
                            start=(g == 0),
                            stop=(g == CB - 1),
                        )
                    nc.vector.tensor_add(
                        o_sb[0:1, ch * 512 : (ch + 1) * 512],
                        o_ps[0:1, :],
                        bproj_sb[0:1, ch * 512 : (ch + 1) * 512],
                    )
                nc.sync.dma_start(out_ap[b : b + 1, :], o_sb[:])

            def emit_wvwp():
                """Stage-load Wv (sync) / Wproj (scalar) f32, cast on ACT."""
                for g in range(CB):
                    for dst, src_ap, c0, eng in (
                        (wv_bf, wkv_ap, 1024, nc.sync),
                        (wp_bf, wp_ap, 0, nc.scalar),
                    ):
                        wst = wstage.tile([128, 1024], F32, tag="wst")
                        eng.dma_start(
                            wst[:],
                            src_ap[g * 128 : (g + 1) * 128, c0 : c0 + 1024],
                        )
                        nc.scalar.copy(dst[:, g, :], wst[:])

            # ---- global schedule ----
            sweep_chunks(0, range(NCH))
            emit_racc(0)                # drain chunk 3
            fin_part1(0)
            sweep_chunks(1, [0, 1])
            emit_wvwp()
            fin_part2(0)
            sweep_chunks(1, [2, 3])
            emit_racc(1)
            fin_part1(1)
            fin_part2(1)

            t16_psum.__exit__(None, None, None)
            xat_psum.__exit__(None, None, None)
            sweep_psum.__exit__(None, None, None)


_CACHED = None


def _get_program():
    global _CACHED
    if _CACHED is None:
        _CACHED = _build()
    return _CACHED


def kernel(x, Wq, Wkv, Wproj, bproj, _trace=False):
    x = np.ascontiguousarray(np.asarray(x, dtype=np.float32))
    Wq = np.ascontiguousarray(np.asarray(Wq, dtype=np.float32))
    Wkv = np.ascontiguousarray(np.asarray(Wkv, dtype=np.float32))
    Wproj = np.ascontiguousarray(np.asarray(Wproj, dtype=np.float32))
    bproj = np.ascontiguousarray(np.asarray(bproj, dtype=np.float32))

    nc = _get_program()
    in_maps = [
        {
            "x": x[cid * BPC : (cid + 1) * BPC],
            "Wq": Wq,
            "Wkv": Wkv,
            "Wproj": Wproj,
            "bproj": bproj,
        }
        for cid in range(N_CORES)
    ]
    res = run_bass_kernel_spmd(
        nc, in_maps, core_ids=list(range(N_CORES)), trace=_trace
    )
    out = np.concatenate([res.results[cid]["out"] for cid in range(N_CORES)], axis=0)
    if _trace:
        kernel.last_exec_time_ns = res.exec_time_ns
        kernel.last_results = res
    return out.reshape(B, 1, C)
